# revision 7
# baseline (speedup 1.0000x reference)
"""EntityNLM forward on 8 trn2 NeuronCores.

Sharding: core c handles document b = c // 2 and vocab half vh = c % 2.
Every core runs the sequential entity-LSTM recurrence for its document
(entity indices arrive as per-core int32 offset arrays, consumed via engine
registers + dynamic access patterns), then computes its (doc, vocab-half)
slice of the logits matmul, streaming results to DRAM. Host reassembles.
No collectives.

Self-contained: shapes hardcoded; only numpy + concourse imports.
"""
import numpy as np

V, D, H, E, B, T = 50257, 300, 128, 64, 4, 512
DPAD = 384            # D padded to 3x128 for K-chunked matmuls
VPAD = 51200          # V padded to 8x6400; halves of 25600 = 50 blocks of 512
VH = VPAD // 2
NVB = VH // 512       # vocab blocks per half (50)


def _host_prep(inputs):
    """Index/layout plumbing on host (int bookkeeping + layout transforms)."""
    f32 = np.float32
    tokens = np.asarray(inputs["tokens"]).astype(np.int64)
    eid = np.asarray(inputs["entity_ids"]).astype(np.int64)
    sid = np.asarray(inputs["sent_ids"]).astype(np.float64)
    embed = np.asarray(inputs["embed"], f32)
    Wih = np.asarray(inputs["Wih"], f32)
    Whh = np.asarray(inputs["Whh"], f32)
    b2v = (np.asarray(inputs["bih"]) + np.asarray(inputs["bhh"])).astype(f32)
    We = np.asarray(inputs["We"], f32); Wd = np.asarray(inputs["Wd"], f32)
    WTe = np.asarray(inputs["WTe"], f32); WTc = np.asarray(inputs["WTc"], f32)
    Wx = np.asarray(inputs["Wx"], f32)
    e0 = np.asarray(inputs["ents0"], f32)
    lam = float(np.asarray(inputs["lam"]))

    GORD = [0, 1, 3, 2]   # torch gate order i,f,g,o -> device order i,f,o,g

    WihT = np.zeros((DPAD, 4 * H), f32)
    Wih4 = Wih.reshape(4, H, D)
    for gi, g in enumerate(GORD):
        WihT[:D, gi * H:(gi + 1) * H] = Wih4[g].T
    b2 = np.stack([b2v.reshape(4, H)[g] for g in GORD], 1)        # (H,4)

    WhhT = np.zeros((H, 4 * H), f32)
    Whh4 = Whh.reshape(4, H, H)
    for gi, g in enumerate(GORD):
        WhhT[:, gi * H:(gi + 1) * H] = Whh4[g].T

    WxT = np.zeros((H, VPAD), f32)
    WxT[:, :V] = Wx.T

    shared = dict(
        WihT=WihT, b2=b2, WhhT=WhhT,
        WeT=np.ascontiguousarray(We.T), WdT=np.ascontiguousarray(Wd.T),
        WTeT=np.ascontiguousarray(WTe.T), WTcT=np.ascontiguousarray(WTc.T),
        Wr=np.asarray(inputs["Wr"], f32).reshape(H, 1),
        ident=np.eye(H, dtype=f32),
    )

    percore = []
    for b in range(B):
        SM = np.zeros((E, T), f32)
        dists = np.zeros(E, np.float64)
        for t in range(T):
            SM[:, t] = dists - sid[b, t]
            dists[eid[b, t]] = sid[b, t]
        EtokT = np.zeros((DPAD, T), f32)
        EtokT[:D] = embed[tokens[b]].T
        percore.append(dict(
            EtokT=EtokT,
            E0C=np.ascontiguousarray(e0[b].T),
            E0R=e0[b].reshape(1, E * H).astype(f32),
            eidx=eid[b].reshape(1, T).astype(np.int32),
            eoff=(eid[b] * H).reshape(1, T).astype(np.int32),
            SM=SM,
            mask=np.broadcast_to((eid[b] > 0).astype(f32), (H, T)).copy(),
        ))

    return shared, percore, WxT, lam, float(np.asarray(inputs["br"])), \
        np.asarray(inputs["bx"], f32)


def _build_program(lam, br, nsteps=T):
    import concourse.bacc as bacc
    import concourse.mybir as mybir
    import concourse.tile as tile
    import concourse.bass as bass

    dt = mybir.dt
    AF = mybir.ActivationFunctionType
    OP = mybir.AluOpType

    nc = bacc.Bacc("TRN2", target_bir_lowering=False)

    def din(name, shape, dtype=dt.float32):
        return nc.dram_tensor(name, shape, dtype, kind="ExternalInput")

    EtokT = din("EtokT", [DPAD, T]); WihT = din("WihT", [DPAD, 4 * H])
    b2 = din("b2", [H, 4]); WhhT = din("WhhT", [H, 4 * H])
    WeT = din("WeT", [H, H]); WdT = din("WdT", [H, H])
    WTeT = din("WTeT", [H, H]); WTcT = din("WTcT", [H, H])
    Wr = din("Wr", [H, 1]); ident = din("ident", [H, H])
    E0C = din("E0C", [H, E]); E0R = din("E0R", [1, E * H])
    eidx = din("eidx", [1, T], dt.int32); eoff = din("eoff", [1, T], dt.int32)
    SMd = din("SM", [E, T]); maskd = din("mask", [H, T])
    WxTd = din("WxT", [H, VH], dt.float16)
    out_logits = nc.dram_tensor("out_logits", [T, VH], dt.float32,
                                kind="ExternalOutput")
    out_pr = nc.dram_tensor("out_pr", [1, T], dt.float32, kind="ExternalOutput")
    out_pe = nc.dram_tensor("out_pe", [E, T], dt.float32, kind="ExternalOutput")

    with tile.TileContext(nc) as tc:
        with (
            tc.tile_pool(name="cst", bufs=1) as cst,
            tc.tile_pool(name="loop", bufs=4) as lp,
            tc.tile_pool(name="str", bufs=3) as strm,
            tc.tile_pool(name="pg", bufs=2, space="PSUM") as pg,
            tc.tile_pool(name="prow", bufs=2, space="PSUM") as prow,
            tc.tile_pool(name="pcol", bufs=2, space="PSUM") as pcol,
            tc.tile_pool(name="pbig", bufs=2, space="PSUM") as pbig,
        ):
            f = dt.float32
            fr = dt.float32r

            def load(dram, shape, dtype=f):
                t_ = cst.tile(shape, dtype, tag=f"ld_{dram.name}")
                nc.sync.dma_start(t_[:], dram[:])
                return t_

            # persistent SBUF (DPAD-partition tensors load as (128, 3, cols))
            etok = cst.tile([H, 3 * T], f)
            nc.sync.dma_start(etok[:].rearrange("p (c t) -> p c t", c=3),
                              EtokT[:].rearrange("(c p) t -> p c t", p=H))
            wih = cst.tile([H, 3 * 4 * H], f)
            nc.sync.dma_start(wih[:].rearrange("p (c u) -> p c u", c=3),
                              WihT[:].rearrange("(c p) u -> p c u", p=H))
            b2s = load(b2, [H, 4]); whh = load(WhhT, [H, 4 * H])
            wes = load(WeT, [H, H]); wds = load(WdT, [H, H])
            wtes = load(WTeT, [H, H]); wtcs = load(WTcT, [H, H])
            wrs = load(Wr, [H, 1]); ids = load(ident, [H, H])
            bankC = load(E0C, [H, E])
            bankR = load(E0R, [1, E * H])
            eix = load(eidx, [1, T], dt.int32)
            eox = load(eoff, [1, T], dt.int32)
            sms = load(SMd, [E, T]); msk = load(maskd, [H, T])

            XP = cst.tile([H, 4 * T], f)        # col g*T + t
            H4 = cst.tile([H, T], f)
            UC = cst.tile([H, 1 + T], f)
            DC = cst.tile([H, T], f)
            GC = cst.tile([H, T], f)
            PEH = cst.tile([E, T], f)
            ZC = cst.tile([H, T], f)
            cstate = cst.tile([H, 1], f)
            nc.vector.memset(cstate[:], 0.0)
            h0 = cst.tile([H, 1], f)
            nc.vector.memset(h0[:], 0.0)
            nc.scalar.copy(UC[:, 0:1], bankC[:, 0:1])

            etok_v = etok[:].rearrange("p (c t) -> p c t", c=3)
            wih_v = wih[:].rearrange("p (c u) -> p c u", c=3)

            # XP = WihT.T @ EtokT + b2, per gate
            for g in range(4):
                ps = pbig.tile([H, T], f, tag="big")
                for dc in range(3):
                    nc.tensor.matmul(
                        ps[:],
                        wih_v[:, dc, g * H:(g + 1) * H],
                        etok_v[:, dc, :],
                        start=(dc == 0), stop=(dc == 2))
                nc.scalar.activation(XP[:, g * T:(g + 1) * T], ps[:], AF.Identity,
                                     bias=b2s[:, g:g + 1])

            xp_v = XP[:].rearrange("p (g t) -> p g t", g=4)

            hprev = h0
            for t in range(nsteps):
                # ---- LSTM cell (order i,f,o,g)
                gps = pg.tile([H, 4], f, tag="gates")
                for g in range(4):
                    nc.tensor.matmul(gps[:, g:g + 1], whh[:, g * H:(g + 1) * H],
                                     hprev[:], start=True, stop=True)
                gsb = lp.tile([H, 4], f, tag="gsb")
                nc.vector.tensor_add(gsb[:], gps[:], xp_v[:, :, t])
                act = lp.tile([H, 4], f, tag="act")
                nc.scalar.activation(act[:, 0:3], gsb[:, 0:3], AF.Sigmoid)
                nc.scalar.activation(act[:, 3:4], gsb[:, 3:4], AF.Tanh)
                m1 = lp.tile([H, 1], f, tag="m1")
                nc.vector.tensor_mul(m1[:], act[:, 0:1], act[:, 3:4])
                nc.vector.scalar_tensor_tensor(cstate[:], cstate[:], act[:, 1:2],
                                               m1[:], OP.mult, OP.add)
                tct = lp.tile([H, 1], f, tag="tc")
                nc.scalar.activation(tct[:], cstate[:], AF.Tanh)
                hcol = H4[:, t:t + 1]
                nc.vector.tensor_mul(hcol, act[:, 2:3], tct[:])
                hprev = hcol

                # ---- entity machinery
                rowp = prow.tile([1, 136], f, tag="rowp")   # [0:128] hrow, [128] dot
                nc.tensor.matmul(rowp[:, 0:H], hcol, ids[:], is_transpose=True,
                                 start=True, stop=True)

                rpe = nc.tensor.alloc_register(f"rpe{t}")
                nc.tensor.reg_load(rpe, eix[0:1, t:t + 1])
                o_pe = nc.tensor.snap(rpe, donate=True, min_val=0, max_val=E - 1)
                rdv = nc.vector.alloc_register(f"rdv{t}")
                nc.vector.reg_load(rdv, eox[0:1, t:t + 1])
                o_dv = nc.vector.snap(rdv, donate=True, min_val=0,
                                      max_val=(E - 1) * H)
                rac = nc.scalar.alloc_register(f"rac{t}")
                nc.scalar.reg_load(rac, eix[0:1, t:t + 1])
                o_ac = nc.scalar.snap(rac, donate=True, min_val=0, max_val=E - 1)

                colp = pcol.tile([H, 4], f, tag="colp")     # d | g | ucol | pe(64)
                nc.tensor.matmul(colp[:, 0:1], wds[:], hcol, start=True, stop=True)
                nc.tensor.matmul(colp[:, 1:2], wes[:], hcol, start=True, stop=True)
                nc.scalar.copy(DC[:, t:t + 1], colp[:, 0:1])
                nc.scalar.copy(GC[:, t:t + 1], colp[:, 1:2])

                nc.tensor.matmul(rowp[:, H:H + 1], DC[:, t:t + 1],
                                 bankC[:, bass.ds(o_pe, 1)], start=True, stop=True)
                delta = lp.tile([1, 1], f, tag="delta")
                nc.scalar.activation(delta[:], rowp[:, H:H + 1], AF.Sigmoid)

                diff = lp.tile([1, H], f, tag="diff")
                nc.vector.tensor_sub(diff[:], bankR[0:1, bass.ds(o_dv, H)],
                                     rowp[:, 0:H])
                upd = lp.tile([1, H], f, tag="upd")
                nc.vector.scalar_tensor_tensor(upd[:], diff[:], delta[:],
                                               rowp[:, 0:H], OP.mult, OP.add)
                sq = lp.tile([1, 1], f, tag="sq")
                sqt = lp.tile([1, H], f, tag="sqt")
                nc.scalar.activation(sqt[:], upd[:], AF.Square, accum_out=sq[:])
                rt = lp.tile([1, 1], f, tag="rt")
                nc.scalar.sqrt(rt[:], sq[:])
                rln = lp.tile([1, 1], f, tag="rln")
                nc.vector.reciprocal(rln[:], rt[:])
                updn = lp.tile([1, H], f, tag="updn")
                nc.vector.tensor_scalar_mul(updn[:], upd[:], rln[:])
                nc.vector.tensor_copy(bankR[0:1, bass.ds(o_dv, H)], updn[:])
                nc.tensor.matmul(colp[:, 2:3], updn[:], ids[0:1, 0:1],
                                 is_transpose=True, start=True, stop=True)
                nc.scalar.copy(UC[:, 1 + t:2 + t], colp[:, 2:3])

                # pred_e raw BEFORE this step's bankC scatter
                nc.tensor.matmul(colp[0:E, 3:4], bankC[:], GC[:, t:t + 1],
                                 start=True, stop=True)
                nc.scalar.copy(PEH[:, t:t + 1], colp[0:E, 3:4])
                nc.scalar.copy(bankC[:, bass.ds(o_ac, 1)], colp[:, 2:3])

                # ---- per-chunk tails: ent_feat, Z, logits
                if (t + 1) % 128 == 0:
                    s = (t // 128) * 128
                    efe = pbig.tile([H, 128], f, tag="big")
                    nc.tensor.matmul(efe[:], wtes[:], UC[:, 1 + s:1 + s + 128],
                                     start=True, stop=True)
                    efc = pbig.tile([H, 128], f, tag="big")
                    nc.tensor.matmul(efc[:], wtcs[:], UC[:, s:s + 128],
                                     start=True, stop=True)
                    efcs = lp.tile([H, 128], f, tag="efcs")
                    nc.scalar.copy(efcs[:], efc[:])
                    zc = ZC[:, s:s + 128]
                    nc.vector.tensor_sub(zc, efe[:], efcs[:])
                    nc.vector.tensor_mul(zc, zc, msk[:, s:s + 128])
                    nc.vector.tensor_add(zc, zc, efcs[:])
                    nc.vector.tensor_add(zc, zc, H4[:, s:s + 128])
                    zc16 = lp.tile([H, 128], dt.float16, tag="zc16")
                    nc.vector.tensor_copy(zc16[:], zc)
                    for vb in range(NVB):
                        wx = strm.tile([H, 512], dt.float16, tag="wx")
                        nc.sync.dma_start(wx[:], WxTd[:, vb * 512:(vb + 1) * 512])
                        lps = pbig.tile([128, 512], f, tag="big")
                        nc.tensor.matmul(lps[:], zc16[:], wx[:],
                                         start=True, stop=True)
                        lsb = strm.tile([128, 512], f, tag="lsb")
                        if vb % 2 == 0:
                            nc.scalar.copy(lsb[:], lps[:])
                        else:
                            nc.vector.tensor_copy(lsb[:], lps[:])
                        nc.sync.dma_start(
                            out_logits[s:s + 128, vb * 512:(vb + 1) * 512], lsb[:])

            # pred_r
            prps = pbig.tile([1, T], f, tag="big")
            nc.tensor.matmul(prps[:], wrs[:], H4[:], start=True, stop=True)
            prsb = cst.tile([1, T], f)
            nc.scalar.activation(prsb[:], prps[:], AF.Sigmoid, bias=br)
            nc.sync.dma_start(out_pr[:], prsb[:])

            # pred_e = raw + exp(lam * SM)
            esm = cst.tile([E, T], f)
            nc.scalar.activation(esm[:], sms[:], AF.Exp, scale=lam)
            peo = cst.tile([E, T], f)
            nc.vector.tensor_add(peo[:], PEH[:], esm[:])
            nc.sync.dma_start(out_pe[:], peo[:])

    nc.compile()
    return nc


_prog_cache = {}


def kernel(**inputs):
    from concourse.bass_utils import run_bass_kernel_spmd

    shared, percore, WxT, lam, br, bx = _host_prep(inputs)

    if "prog" not in _prog_cache:
        _prog_cache["prog"] = _build_program(lam, br)
    nc = _prog_cache["prog"]

    in_maps = []
    for c in range(8):
        b, vh = c // 2, c % 2
        m = dict(shared)
        m.update(percore[b])
        m["WxT"] = np.ascontiguousarray(WxT[:, vh * VH:(vh + 1) * VH]).astype(np.float16)
        in_maps.append(m)

    res = run_bass_kernel_spmd(nc, in_maps, core_ids=list(range(8)))
    globals()["LAST_EXEC_NS"] = res.exec_time_ns

    f32 = np.float32
    logits = np.empty((B, T, V), f32)
    pred_r = np.empty((B, T), f32)
    pred_e = np.empty((B, T, E), f32)
    for c in range(8):
        b, vh = c // 2, c % 2
        lo = vh * VH
        hi = min(V, lo + VH)
        if hi > lo:
            logits[b, :, lo:hi] = res.results[c]["out_logits"][:, :hi - lo]
        if vh == 0:
            pred_r[b] = res.results[c]["out_pr"][0]
            pred_e[b] = res.results[c]["out_pe"].T
    logits += bx[None, None, :]
    return logits, pred_r, pred_e


# revision 13
# speedup vs baseline: 3.0850x; 3.0850x over previous
"""EntityNLM forward on 8 trn2 NeuronCores.

Sharding: core c handles document b = c // 2 and vocab half vh = c % 2.
Every core runs the sequential entity-LSTM recurrence for its document
(entity indices arrive as per-core int32 offset arrays, consumed via engine
registers + dynamic access patterns), then computes its (doc, vocab-half)
slice of the logits matmul, streaming results to DRAM. Host reassembles.
No collectives.

Self-contained: shapes hardcoded; only numpy + concourse imports.
"""
import numpy as np

V, D, H, E, B, T = 50257, 300, 128, 64, 4, 512
DPAD = 384            # D padded to 3x128 for K-chunked matmuls
VPAD = 51200          # V padded to 8x6400; halves of 25600 = 50 blocks of 512
VH = VPAD // 2
NVB = VH // 512       # vocab blocks per half (50)


def _host_prep(inputs):
    """Index/layout plumbing on host (int bookkeeping + layout transforms)."""
    f32 = np.float32
    tokens = np.asarray(inputs["tokens"]).astype(np.int64)
    eid = np.asarray(inputs["entity_ids"]).astype(np.int64)
    sid = np.asarray(inputs["sent_ids"]).astype(np.float64)
    embed = np.asarray(inputs["embed"], f32)
    Wih = np.asarray(inputs["Wih"], f32)
    Whh = np.asarray(inputs["Whh"], f32)
    b2v = (np.asarray(inputs["bih"]) + np.asarray(inputs["bhh"])).astype(f32)
    We = np.asarray(inputs["We"], f32); Wd = np.asarray(inputs["Wd"], f32)
    WTe = np.asarray(inputs["WTe"], f32); WTc = np.asarray(inputs["WTc"], f32)
    Wx = np.asarray(inputs["Wx"], f32)
    e0 = np.asarray(inputs["ents0"], f32)
    lam = float(np.asarray(inputs["lam"]))

    GORD = [0, 1, 3, 2]   # torch gate order i,f,g,o -> device order i,f,o,g

    WihT = np.zeros((DPAD, 4 * H), f32)
    Wih4 = Wih.reshape(4, H, D)
    for gi, g in enumerate(GORD):
        WihT[:D, gi * H:(gi + 1) * H] = Wih4[g].T
    b2 = np.stack([b2v.reshape(4, H)[g] for g in GORD], 1)        # (H,4)

    WhhT = np.zeros((H, 4 * H), f32)
    Whh4 = Whh.reshape(4, H, H)
    for gi, g in enumerate(GORD):
        WhhT[:, gi * H:(gi + 1) * H] = Whh4[g].T

    WxT = np.zeros((H, VPAD), f32)
    WxT[:, :V] = Wx.T

    shared = dict(
        WihT=WihT, b2=b2, WhhT=WhhT,
        WeT=np.ascontiguousarray(We.T), WdT=np.ascontiguousarray(Wd.T),
        WTeT=np.ascontiguousarray(WTe.T), WTcT=np.ascontiguousarray(WTc.T),
        Wr=np.asarray(inputs["Wr"], f32).reshape(H, 1),
        ident=np.eye(H, dtype=f32),
    )

    percore = []
    for b in range(B):
        SM = np.zeros((E, T), f32)
        dists = np.zeros(E, np.float64)
        for t in range(T):
            SM[:, t] = dists - sid[b, t]
            dists[eid[b, t]] = sid[b, t]
        EtokT = np.zeros((DPAD, T), f32)
        EtokT[:D] = embed[tokens[b]].T
        percore.append(dict(
            EtokT=EtokT,
            E0C=np.ascontiguousarray(e0[b].T),
            E0R=e0[b].reshape(1, E * H).astype(f32),
            eidx=eid[b].reshape(1, T).astype(np.int32),
            eoff=(eid[b] * H).reshape(1, T).astype(np.int32),
            SM=SM,
            mask=np.broadcast_to((eid[b] > 0).astype(f32), (H, T)).copy(),
        ))

    return shared, percore, WxT, lam, float(np.asarray(inputs["br"])), \
        np.asarray(inputs["bx"], f32)


def _build_program(lam, br, nsteps=T, skip_b=False, skip_c=False):
    import concourse.bacc as bacc
    import concourse.mybir as mybir
    import concourse.tile as tile
    import concourse.bass as bass

    dt = mybir.dt
    AF = mybir.ActivationFunctionType
    OP = mybir.AluOpType

    nc = bacc.Bacc("TRN2", target_bir_lowering=False)

    def din(name, shape, dtype=dt.float32):
        return nc.dram_tensor(name, shape, dtype, kind="ExternalInput")

    EtokT = din("EtokT", [DPAD, T]); WihT = din("WihT", [DPAD, 4 * H])
    b2 = din("b2", [H, 4]); WhhT = din("WhhT", [H, 4 * H])
    WeT = din("WeT", [H, H]); WdT = din("WdT", [H, H])
    WTeT = din("WTeT", [H, H]); WTcT = din("WTcT", [H, H])
    Wr = din("Wr", [H, 1]); ident = din("ident", [H, H])
    E0C = din("E0C", [H, E]); E0R = din("E0R", [1, E * H])
    eidx = din("eidx", [1, T], dt.int32); eoff = din("eoff", [1, T], dt.int32)
    SMd = din("SM", [E, T]); maskd = din("mask", [H, T])
    WxTd = din("WxT", [H, VH], dt.float16)
    out_logits = nc.dram_tensor("out_logits", [T, VH], dt.float32,
                                kind="ExternalOutput")
    out_pr = nc.dram_tensor("out_pr", [1, T], dt.float32, kind="ExternalOutput")
    out_pe = nc.dram_tensor("out_pe", [E, T], dt.float32, kind="ExternalOutput")

    with tile.TileContext(nc) as tc:
        with (
            tc.tile_pool(name="cst", bufs=1) as cst,
            tc.tile_pool(name="loop", bufs=6) as lp,
            tc.tile_pool(name="str", bufs=3) as strm,
            tc.tile_pool(name="pg", bufs=2, space="PSUM") as pg,
            tc.tile_pool(name="prow", bufs=2, space="PSUM") as prow,
            tc.tile_pool(name="pcol", bufs=2, space="PSUM") as pcol,
            tc.tile_pool(name="pbig", bufs=2, space="PSUM") as pbig,
        ):
            f = dt.float32
            fr = dt.float32r

            def load(dram, shape, dtype=f):
                t_ = cst.tile(shape, dtype, tag=f"ld_{dram.name}")
                nc.sync.dma_start(t_[:], dram[:])
                return t_

            # persistent SBUF (DPAD-partition tensors load as (128, 3, cols))
            etok = cst.tile([H, 3 * T], f)
            nc.sync.dma_start(etok[:].rearrange("p (c t) -> p c t", c=3),
                              EtokT[:].rearrange("(c p) t -> p c t", p=H))
            wih = cst.tile([H, 3 * 4 * H], f)
            nc.sync.dma_start(wih[:].rearrange("p (c u) -> p c u", c=3),
                              WihT[:].rearrange("(c p) u -> p c u", p=H))
            b2s = load(b2, [H, 4]); whh = load(WhhT, [H, 4 * H])
            wes = load(WeT, [H, H]); wds = load(WdT, [H, H])
            wtes = load(WTeT, [H, H]); wtcs = load(WTcT, [H, H])
            wrs = load(Wr, [H, 1]); ids = load(ident, [H, H])
            bankC = load(E0C, [H, E])
            bankR = load(E0R, [1, E * H])
            eix = load(eidx, [1, T], dt.int32)
            eox = load(eoff, [1, T], dt.int32)
            sms = load(SMd, [E, T]); msk = load(maskd, [H, T])

            XP = cst.tile([H, 4 * T], f)        # col g*T + t
            H4 = cst.tile([H, T], f)
            UC = cst.tile([H, 1 + T], f)
            DC = cst.tile([H, T], f)
            GC = cst.tile([H, T], f)
            PEH = cst.tile([E, T], f)
            ZC = cst.tile([H, T], f)
            cstate = cst.tile([H, 1], f)
            nc.vector.memset(cstate[:], 0.0)
            h0 = cst.tile([H, 1], f)
            nc.vector.memset(h0[:], 0.0)
            nc.scalar.copy(UC[:, 0:1], bankC[:, 0:1])

            etok_v = etok[:].rearrange("p (c t) -> p c t", c=3)
            wih_v = wih[:].rearrange("p (c u) -> p c u", c=3)

            # XP = WihT.T @ EtokT + b2, per gate
            for g in range(4):
                ps = pbig.tile([H, T], f, tag="big")
                for dc in range(3):
                    nc.tensor.matmul(
                        ps[:],
                        wih_v[:, dc, g * H:(g + 1) * H],
                        etok_v[:, dc, :],
                        start=(dc == 0), stop=(dc == 2))
                nc.scalar.activation(XP[:, g * T:(g + 1) * T], ps[:], AF.Identity,
                                     bias=b2s[:, g:g + 1])

            xp_v = XP[:].rearrange("p (g t) -> p g t", g=4)

            def emit_A(t, hprev):
                # ---- LSTM cell (order i,f,o,g)
                gps = pg.tile([H, 4], f, tag="gates")
                for g in range(4):
                    nc.tensor.matmul(gps[:, g:g + 1], whh[:, g * H:(g + 1) * H],
                                     hprev[:], start=True, stop=True)
                gsb = lp.tile([H, 4], f, tag="gsb")
                nc.vector.tensor_add(gsb[:], gps[:], xp_v[:, :, t])
                act = lp.tile([H, 4], f, tag="act")
                nc.scalar.activation(act[:, 0:3], gsb[:, 0:3], AF.Sigmoid)
                nc.scalar.activation(act[:, 3:4], gsb[:, 3:4], AF.Tanh)
                m1 = lp.tile([H, 1], f, tag="m1")
                nc.vector.tensor_mul(m1[:], act[:, 0:1], act[:, 3:4])
                nc.vector.scalar_tensor_tensor(cstate[:], cstate[:], act[:, 1:2],
                                               m1[:], OP.mult, OP.add)
                tct = lp.tile([H, 1], f, tag="tc")
                nc.scalar.activation(tct[:], cstate[:], AF.Tanh)
                hcol = H4[:, t:t + 1]
                nc.vector.tensor_mul(hcol, act[:, 2:3], tct[:])
                return hcol

            def emit_B(t):
                hcol = H4[:, t:t + 1]
                rowp = prow.tile([1, 136], f, tag="rowp")   # [0:128] hrow, [128] dot
                nc.tensor.matmul(rowp[:, 0:H], hcol, ids[:], is_transpose=True,
                                 start=True, stop=True)

                rpe = nc.tensor.alloc_register(f"rpe{t}")
                nc.tensor.reg_load(rpe, eix[0:1, t:t + 1])
                o_pe = nc.tensor.snap(rpe, donate=True, min_val=0, max_val=E - 1)
                rdv = nc.vector.alloc_register(f"rdv{t}")
                nc.vector.reg_load(rdv, eox[0:1, t:t + 1])
                o_dv = nc.vector.snap(rdv, donate=True, min_val=0,
                                      max_val=(E - 1) * H)
                rac = nc.scalar.alloc_register(f"rac{t}")
                nc.scalar.reg_load(rac, eix[0:1, t:t + 1])
                o_ac = nc.scalar.snap(rac, donate=True, min_val=0, max_val=E - 1)

                colp = pcol.tile([H, 4], f, tag="colp")     # d | g | ucol | pe(64)
                nc.tensor.matmul(colp[:, 0:1], wds[:], hcol, start=True, stop=True)
                nc.tensor.matmul(colp[:, 1:2], wes[:], hcol, start=True, stop=True)
                nc.scalar.copy(DC[:, t:t + 1], colp[:, 0:1])
                nc.vector.tensor_copy(GC[:, t:t + 1], colp[:, 1:2])

                nc.tensor.matmul(rowp[:, H:H + 1], DC[:, t:t + 1],
                                 bankC[:, bass.ds(o_pe, 1)], start=True, stop=True)
                delta = lp.tile([1, 1], f, tag="delta")
                nc.scalar.activation(delta[:], rowp[:, H:H + 1], AF.Sigmoid)

                diff = lp.tile([1, H], f, tag="diff")
                nc.vector.tensor_sub(diff[:], bankR[0:1, bass.ds(o_dv, H)],
                                     rowp[:, 0:H])
                upd = lp.tile([1, H], f, tag="upd")
                nc.vector.scalar_tensor_tensor(upd[:], diff[:], delta[:],
                                               rowp[:, 0:H], OP.mult, OP.add)
                sq = lp.tile([1, 1], f, tag="sq")
                sqt = lp.tile([1, H], f, tag="sqt")
                nc.scalar.activation(sqt[:], upd[:], AF.Square, accum_out=sq[:])
                rt = lp.tile([1, 1], f, tag="rt")
                nc.scalar.sqrt(rt[:], sq[:])
                rln = lp.tile([1, 1], f, tag="rln")
                nc.vector.reciprocal(rln[:], rt[:])
                updn = lp.tile([1, H], f, tag="updn")
                nc.vector.tensor_scalar_mul(updn[:], upd[:], rln[:])
                nc.vector.tensor_copy(bankR[0:1, bass.ds(o_dv, H)], updn[:])
                nc.tensor.matmul(colp[:, 2:3], updn[:], ids[0:1, 0:1],
                                 is_transpose=True, start=True, stop=True)
                nc.scalar.copy(UC[:, 1 + t:2 + t], colp[:, 2:3])

                # pred_e raw BEFORE this step's bankC scatter
                nc.tensor.matmul(colp[0:E, 3:4], bankC[:], GC[:, t:t + 1],
                                 start=True, stop=True)
                nc.scalar.copy(PEH[:, t:t + 1], colp[0:E, 3:4])
                nc.scalar.copy(bankC[:, bass.ds(o_ac, 1)], colp[:, 2:3])

            def emit_chunk(s):
                # ---- per-chunk tails: ent_feat, Z, logits
                if True:
                    efe = pbig.tile([H, 128], f, tag="big")
                    nc.tensor.matmul(efe[:], wtes[:], UC[:, 1 + s:1 + s + 128],
                                     start=True, stop=True)
                    efc = pbig.tile([H, 128], f, tag="big")
                    nc.tensor.matmul(efc[:], wtcs[:], UC[:, s:s + 128],
                                     start=True, stop=True)
                    efcs = lp.tile([H, 128], f, tag="efcs")
                    nc.scalar.copy(efcs[:], efc[:])
                    zc = ZC[:, s:s + 128]
                    nc.vector.tensor_sub(zc, efe[:], efcs[:])
                    nc.vector.tensor_mul(zc, zc, msk[:, s:s + 128])
                    nc.vector.tensor_add(zc, zc, efcs[:])
                    nc.vector.tensor_add(zc, zc, H4[:, s:s + 128])
                    zc16 = lp.tile([H, 128], dt.float16, tag="zc16")
                    nc.vector.tensor_copy(zc16[:], zc)
                    for vb in range(NVB):
                        wx = strm.tile([H, 512], dt.float16, tag="wx")
                        nc.sync.dma_start(wx[:], WxTd[:, vb * 512:(vb + 1) * 512])
                        lps = pbig.tile([128, 512], f, tag="big")
                        nc.tensor.matmul(lps[:], zc16[:], wx[:],
                                         start=True, stop=True)
                        lsb = strm.tile([128, 512], f, tag="lsb")
                        if vb % 2 == 0:
                            nc.scalar.copy(lsb[:], lps[:])
                        else:
                            nc.vector.tensor_copy(lsb[:], lps[:])
                        nc.sync.dma_start(
                            out_logits[s:s + 128, vb * 512:(vb + 1) * 512], lsb[:])

            # software-pipelined emission: A(t) ahead of B(t-1)
            hprev = h0
            for t in range(nsteps):
                hprev = emit_A(t, hprev)
                if not skip_b and t >= 1:
                    emit_B(t - 1)
                    if t % 128 == 0 and not skip_c:
                        emit_chunk(t - 128)
            if not skip_b:
                emit_B(nsteps - 1)
            if not skip_c:
                emit_chunk(nsteps - 128)

            # pred_r
            prps = pbig.tile([1, T], f, tag="big")
            nc.tensor.matmul(prps[:], wrs[:], H4[:], start=True, stop=True)
            prsb = cst.tile([1, T], f)
            nc.scalar.activation(prsb[:], prps[:], AF.Sigmoid, bias=br)
            nc.sync.dma_start(out_pr[:], prsb[:])

            # pred_e = raw + exp(lam * SM)
            esm = cst.tile([E, T], f)
            nc.scalar.activation(esm[:], sms[:], AF.Exp, scale=lam)
            peo = cst.tile([E, T], f)
            nc.vector.tensor_add(peo[:], PEH[:], esm[:])
            nc.sync.dma_start(out_pe[:], peo[:])

    nc.compile()
    return nc


_prog_cache = {}


def kernel(**inputs):
    from concourse.bass_utils import run_bass_kernel_spmd

    shared, percore, WxT, lam, br, bx = _host_prep(inputs)

    if "prog" not in _prog_cache:
        _prog_cache["prog"] = _build_program(lam, br)
    nc = _prog_cache["prog"]

    in_maps = []
    for c in range(8):
        b, vh = c // 2, c % 2
        m = dict(shared)
        m.update(percore[b])
        m["WxT"] = np.ascontiguousarray(WxT[:, vh * VH:(vh + 1) * VH]).astype(np.float16)
        in_maps.append(m)

    res = run_bass_kernel_spmd(nc, in_maps, core_ids=list(range(8)))
    globals()["LAST_EXEC_NS"] = res.exec_time_ns

    f32 = np.float32
    logits = np.empty((B, T, V), f32)
    pred_r = np.empty((B, T), f32)
    pred_e = np.empty((B, T, E), f32)
    for c in range(8):
        b, vh = c // 2, c % 2
        lo = vh * VH
        hi = min(V, lo + VH)
        if hi > lo:
            logits[b, :, lo:hi] = res.results[c]["out_logits"][:, :hi - lo]
        if vh == 0:
            pred_r[b] = res.results[c]["out_pr"][0]
            pred_e[b] = res.results[c]["out_pe"].T
    logits += bx[None, None, :]
    return logits, pred_r, pred_e


# revision 16
# speedup vs baseline: 3.2862x; 1.0652x over previous
"""EntityNLM forward on 8 trn2 NeuronCores.

Sharding: core c handles document b = c // 2 and vocab half vh = c % 2.
Every core runs the sequential entity-LSTM recurrence for its document
(entity indices arrive as per-core int32 offset arrays, consumed via engine
registers + dynamic access patterns), then computes its (doc, vocab-half)
slice of the logits matmul, streaming results to DRAM. Host reassembles.
No collectives.

Self-contained: shapes hardcoded; only numpy + concourse imports.
"""
import numpy as np

V, D, H, E, B, T = 50257, 300, 128, 64, 4, 512
DPAD = 384            # D padded to 3x128 for K-chunked matmuls
VPAD = 51200          # V padded to 8x6400; halves of 25600 = 50 blocks of 512
VH = VPAD // 2
NVB = VH // 512       # vocab blocks per half (50)


def _host_prep(inputs):
    """Index/layout plumbing on host (int bookkeeping + layout transforms)."""
    f32 = np.float32
    tokens = np.asarray(inputs["tokens"]).astype(np.int64)
    eid = np.asarray(inputs["entity_ids"]).astype(np.int64)
    sid = np.asarray(inputs["sent_ids"]).astype(np.float64)
    embed = np.asarray(inputs["embed"], f32)
    Wih = np.asarray(inputs["Wih"], f32)
    Whh = np.asarray(inputs["Whh"], f32)
    b2v = (np.asarray(inputs["bih"]) + np.asarray(inputs["bhh"])).astype(f32)
    We = np.asarray(inputs["We"], f32); Wd = np.asarray(inputs["Wd"], f32)
    WTe = np.asarray(inputs["WTe"], f32); WTc = np.asarray(inputs["WTc"], f32)
    Wx = np.asarray(inputs["Wx"], f32)
    e0 = np.asarray(inputs["ents0"], f32)
    lam = float(np.asarray(inputs["lam"]))

    GORD = [0, 1, 3, 2]   # torch gate order i,f,g,o -> device order i,f,o,g

    WihT = np.zeros((DPAD, 4 * H), f32)
    Wih4 = Wih.reshape(4, H, D)
    for gi, g in enumerate(GORD):
        WihT[:D, gi * H:(gi + 1) * H] = Wih4[g].T
    b2 = np.stack([b2v.reshape(4, H)[g] for g in GORD], 1)        # (H,4)

    WhhT = np.zeros((H, 4 * H), f32)
    Whh4 = Whh.reshape(4, H, H)
    for gi, g in enumerate(GORD):
        WhhT[:, gi * H:(gi + 1) * H] = Whh4[g].T

    WxT = np.zeros((H, VPAD), f32)
    WxT[:, :V] = Wx.T

    shared = dict(
        WihT=WihT, b2=b2, WhhT=WhhT,
        WeT=np.ascontiguousarray(We.T), WdT=np.ascontiguousarray(Wd.T),
        WTeT=np.ascontiguousarray(WTe.T), WTcT=np.ascontiguousarray(WTc.T),
        Wr=np.asarray(inputs["Wr"], f32).reshape(H, 1),
        ident=np.eye(H, dtype=f32),
    )

    percore = []
    for b in range(B):
        SM = np.zeros((E, T), f32)
        dists = np.zeros(E, np.float64)
        for t in range(T):
            SM[:, t] = dists - sid[b, t]
            dists[eid[b, t]] = sid[b, t]
        EtokT = np.zeros((DPAD, T), f32)
        EtokT[:D] = embed[tokens[b]].T
        percore.append(dict(
            EtokT=EtokT,
            E0C=np.ascontiguousarray(e0[b].T),
            E0R=e0[b].reshape(1, E * H).astype(f32),
            eidx=eid[b].reshape(1, T).astype(np.int32),
            eoff=(eid[b] * H).reshape(1, T).astype(np.int32),
            SM=SM,
            mask=np.broadcast_to((eid[b] > 0).astype(f32), (H, T)).copy(),
        ))

    return shared, percore, WxT, lam, float(np.asarray(inputs["br"])), \
        np.asarray(inputs["bx"], f32)


def _build_program(lam, br, nsteps=T, skip_b=False, skip_c=False):
    import concourse.bacc as bacc
    import concourse.mybir as mybir
    import concourse.tile as tile
    import concourse.bass as bass

    dt = mybir.dt
    AF = mybir.ActivationFunctionType
    OP = mybir.AluOpType

    nc = bacc.Bacc("TRN2", target_bir_lowering=False)

    def din(name, shape, dtype=dt.float32):
        return nc.dram_tensor(name, shape, dtype, kind="ExternalInput")

    EtokT = din("EtokT", [DPAD, T]); WihT = din("WihT", [DPAD, 4 * H])
    b2 = din("b2", [H, 4]); WhhT = din("WhhT", [H, 4 * H])
    WeT = din("WeT", [H, H]); WdT = din("WdT", [H, H])
    WTeT = din("WTeT", [H, H]); WTcT = din("WTcT", [H, H])
    Wr = din("Wr", [H, 1]); ident = din("ident", [H, H])
    E0C = din("E0C", [H, E]); E0R = din("E0R", [1, E * H])
    eidx = din("eidx", [1, T], dt.int32); eoff = din("eoff", [1, T], dt.int32)
    SMd = din("SM", [E, T]); maskd = din("mask", [H, T])
    WxTd = din("WxT", [H, VH], dt.float16)
    out_logits = nc.dram_tensor("out_logits", [T, VH], dt.float32,
                                kind="ExternalOutput")
    out_pr = nc.dram_tensor("out_pr", [1, T], dt.float32, kind="ExternalOutput")
    out_pe = nc.dram_tensor("out_pe", [E, T], dt.float32, kind="ExternalOutput")

    with tile.TileContext(nc) as tc:
        with (
            tc.tile_pool(name="cst", bufs=1) as cst,
            tc.tile_pool(name="loop", bufs=6) as lp,
            tc.tile_pool(name="str", bufs=3) as strm,
            tc.tile_pool(name="pg", bufs=2, space="PSUM") as pg,
            tc.tile_pool(name="prow", bufs=2, space="PSUM") as prow,
            tc.tile_pool(name="pcol", bufs=2, space="PSUM") as pcol,
            tc.tile_pool(name="pbig", bufs=2, space="PSUM") as pbig,
        ):
            f = dt.float32
            fr = dt.float32r

            def load(dram, shape, dtype=f):
                t_ = cst.tile(shape, dtype, tag=f"ld_{dram.name}")
                nc.sync.dma_start(t_[:], dram[:])
                return t_

            # persistent SBUF (DPAD-partition tensors load as (128, 3, cols))
            etok = cst.tile([H, 3 * T], f)
            nc.sync.dma_start(etok[:].rearrange("p (c t) -> p c t", c=3),
                              EtokT[:].rearrange("(c p) t -> p c t", p=H))
            wih = cst.tile([H, 3 * 4 * H], f)
            nc.sync.dma_start(wih[:].rearrange("p (c u) -> p c u", c=3),
                              WihT[:].rearrange("(c p) u -> p c u", p=H))
            b2s = load(b2, [H, 4]); whh = load(WhhT, [H, 4 * H])
            wes = load(WeT, [H, H]); wds = load(WdT, [H, H])
            wtes = load(WTeT, [H, H]); wtcs = load(WTcT, [H, H])
            wrs = load(Wr, [H, 1]); ids = load(ident, [H, H])
            bankC = load(E0C, [H, E])
            bankR = load(E0R, [1, E * H])
            eix = load(eidx, [1, T], dt.int32)
            eox = load(eoff, [1, T], dt.int32)
            sms = load(SMd, [E, T]); msk = load(maskd, [H, T])

            XP = cst.tile([H, 4 * T], f)        # col g*T + t
            H4 = cst.tile([H, T], f)
            UC = cst.tile([H, 1 + T], f)
            DG = cst.tile([H, 2 * T], f)   # interleaved d|g columns
            PEH = cst.tile([E, T], f)
            ZC = cst.tile([H, T], f)
            cstate = cst.tile([H, 1], f)
            nc.vector.memset(cstate[:], 0.0)
            h0 = cst.tile([H, 1], f)
            nc.vector.memset(h0[:], 0.0)
            nc.scalar.copy(UC[:, 0:1], bankC[:, 0:1])

            etok_v = etok[:].rearrange("p (c t) -> p c t", c=3)
            wih_v = wih[:].rearrange("p (c u) -> p c u", c=3)

            # XP = WihT.T @ EtokT + b2, per gate
            for g in range(4):
                ps = pbig.tile([H, T], f, tag="big")
                for dc in range(3):
                    nc.tensor.matmul(
                        ps[:],
                        wih_v[:, dc, g * H:(g + 1) * H],
                        etok_v[:, dc, :],
                        start=(dc == 0), stop=(dc == 2))
                nc.scalar.activation(XP[:, g * T:(g + 1) * T], ps[:], AF.Identity,
                                     bias=b2s[:, g:g + 1])

            xp_v = XP[:].rearrange("p (g t) -> p g t", g=4)

            def emit_A(t, hprev):
                # ---- LSTM cell (order i,f,o,g)
                gps = pg.tile([H, 4], f, tag="gates")
                for g in range(4):
                    nc.tensor.matmul(gps[:, g:g + 1], whh[:, g * H:(g + 1) * H],
                                     hprev[:], start=True, stop=True)
                gsb = lp.tile([H, 4], f, tag="gsb")
                nc.vector.tensor_add(gsb[:], gps[:], xp_v[:, :, t])
                act = lp.tile([H, 4], f, tag="act")
                nc.scalar.activation(act[:, 0:3], gsb[:, 0:3], AF.Sigmoid)
                nc.scalar.activation(act[:, 3:4], gsb[:, 3:4], AF.Tanh)
                m1 = lp.tile([H, 1], f, tag="m1")
                nc.vector.tensor_mul(m1[:], act[:, 0:1], act[:, 3:4])
                nc.vector.scalar_tensor_tensor(cstate[:], cstate[:], act[:, 1:2],
                                               m1[:], OP.mult, OP.add)
                tct = lp.tile([H, 1], f, tag="tc")
                nc.scalar.activation(tct[:], cstate[:], AF.Tanh)
                hcol = H4[:, t:t + 1]
                nc.vector.tensor_mul(hcol, act[:, 2:3], tct[:])
                return hcol

            def emit_B(t):
                hcol = H4[:, t:t + 1]
                rowp = prow.tile([1, 136], f, tag="rowp")   # [0:128] hrow, [128] dot
                nc.tensor.matmul(rowp[:, 0:H], hcol, ids[:], is_transpose=True,
                                 start=True, stop=True)

                rpe = nc.tensor.alloc_register(f"rpe{t}")
                nc.tensor.reg_load(rpe, eix[0:1, t:t + 1])
                o_pe = nc.tensor.snap(rpe, donate=True, min_val=0, max_val=E - 1)
                rdv = nc.vector.alloc_register(f"rdv{t}")
                nc.vector.reg_load(rdv, eox[0:1, t:t + 1])
                o_dv = nc.vector.snap(rdv, donate=True, min_val=0,
                                      max_val=(E - 1) * H)
                rac = nc.scalar.alloc_register(f"rac{t}")
                nc.scalar.reg_load(rac, eix[0:1, t:t + 1])
                o_ac = nc.scalar.snap(rac, donate=True, min_val=0, max_val=E - 1)

                colp = pcol.tile([H, 4], f, tag="colp")     # d | g | ucol | pe(64)
                nc.tensor.matmul(colp[:, 0:1], wds[:], hcol, start=True, stop=True)
                nc.tensor.matmul(colp[:, 1:2], wes[:], hcol, start=True, stop=True)
                nc.scalar.copy(DG[:, 2 * t:2 * t + 2], colp[:, 0:2])

                nc.tensor.matmul(rowp[:, H:H + 1], DG[:, 2 * t:2 * t + 1],
                                 bankC[:, bass.ds(o_pe, 1)], start=True, stop=True)
                delta = lp.tile([1, 1], f, tag="delta")
                nc.scalar.activation(delta[:], rowp[:, H:H + 1], AF.Sigmoid)

                diff = lp.tile([1, H], f, tag="diff")
                nc.vector.tensor_sub(diff[:], bankR[0:1, bass.ds(o_dv, H)],
                                     rowp[:, 0:H])
                upd = lp.tile([1, H], f, tag="upd")
                nc.vector.scalar_tensor_tensor(upd[:], diff[:], delta[:],
                                               rowp[:, 0:H], OP.mult, OP.add)
                sq = lp.tile([1, 1], f, tag="sq")
                sqt = lp.tile([1, H], f, tag="sqt")
                nc.scalar.activation(sqt[:], upd[:], AF.Square, accum_out=sq[:])
                # rsqrt via bit-hack + Newton (ACT Sqrt lives in its own table
                # set; switching sets costs ~2.7us per switch, twice per step)
                y0i = lp.tile([1, 1], dt.int32, tag="y0i")
                nc.vector.tensor_scalar(y0i[:], sq[:].bitcast(dt.int32), 1, None,
                                        OP.logical_shift_right)
                nc.vector.tensor_scalar(y0i[:], y0i[:], -1, 0x5F3759DF,
                                        OP.mult, OP.add)
                y = y0i[:].bitcast(f)
                for it in range(2):
                    a_ = lp.tile([1, 1], f, tag=f"nta{it}")
                    nc.scalar.square(a_[:], y)
                    nc.vector.tensor_mul(a_[:], a_[:], sq[:])
                    nc.vector.tensor_scalar(a_[:], a_[:], -0.5, 1.5,
                                            OP.mult, OP.add)
                    yn = lp.tile([1, 1], f, tag=f"ntc{it}")
                    nc.vector.tensor_mul(yn[:], a_[:], y)
                    y = yn[:]
                updn = lp.tile([1, H], f, tag="updn")
                nc.vector.tensor_scalar_mul(updn[:], upd[:], y)
                nc.vector.tensor_copy(bankR[0:1, bass.ds(o_dv, H)], updn[:])
                nc.tensor.matmul(colp[:, 2:3], updn[:], ids[0:1, 0:1],
                                 is_transpose=True, start=True, stop=True)
                nc.scalar.copy(UC[:, 1 + t:2 + t], colp[:, 2:3])

                # pred_e raw BEFORE this step's bankC scatter
                nc.tensor.matmul(colp[0:E, 3:4], bankC[:], DG[:, 2 * t + 1:2 * t + 2],
                                 start=True, stop=True)
                nc.scalar.copy(PEH[:, t:t + 1], colp[0:E, 3:4])
                nc.scalar.copy(bankC[:, bass.ds(o_ac, 1)], colp[:, 2:3])

            def emit_chunk(s):
                # ---- per-chunk tails: ent_feat, Z, logits
                if True:
                    efe = pbig.tile([H, 128], f, tag="big")
                    nc.tensor.matmul(efe[:], wtes[:], UC[:, 1 + s:1 + s + 128],
                                     start=True, stop=True)
                    efc = pbig.tile([H, 128], f, tag="big")
                    nc.tensor.matmul(efc[:], wtcs[:], UC[:, s:s + 128],
                                     start=True, stop=True)
                    efcs = lp.tile([H, 128], f, tag="efcs")
                    nc.scalar.copy(efcs[:], efc[:])
                    zc = ZC[:, s:s + 128]
                    nc.vector.tensor_sub(zc, efe[:], efcs[:])
                    nc.vector.tensor_mul(zc, zc, msk[:, s:s + 128])
                    nc.vector.tensor_add(zc, zc, efcs[:])
                    nc.vector.tensor_add(zc, zc, H4[:, s:s + 128])
                    zc16 = lp.tile([H, 128], dt.float16, tag="zc16")
                    nc.vector.tensor_copy(zc16[:], zc)
                    for vb in range(NVB):
                        wx = strm.tile([H, 512], dt.float16, tag="wx")
                        nc.sync.dma_start(wx[:], WxTd[:, vb * 512:(vb + 1) * 512])
                        lps = pbig.tile([128, 512], f, tag="big")
                        nc.tensor.matmul(lps[:], zc16[:], wx[:],
                                         start=True, stop=True)
                        lsb = strm.tile([128, 512], f, tag="lsb")
                        if vb % 2 == 0:
                            nc.scalar.copy(lsb[:], lps[:])
                        else:
                            nc.vector.tensor_copy(lsb[:], lps[:])
                        nc.sync.dma_start(
                            out_logits[s:s + 128, vb * 512:(vb + 1) * 512], lsb[:])

            # software-pipelined emission: A(t) ahead of B(t-1)
            hprev = h0
            for t in range(nsteps):
                hprev = emit_A(t, hprev)
                if not skip_b and t >= 1:
                    emit_B(t - 1)
                    if t % 128 == 0 and not skip_c:
                        emit_chunk(t - 128)
            if not skip_b:
                emit_B(nsteps - 1)
            if not skip_c:
                emit_chunk(nsteps - 128)

            # pred_r
            prps = pbig.tile([1, T], f, tag="big")
            nc.tensor.matmul(prps[:], wrs[:], H4[:], start=True, stop=True)
            prsb = cst.tile([1, T], f)
            nc.scalar.activation(prsb[:], prps[:], AF.Sigmoid, bias=br)
            nc.sync.dma_start(out_pr[:], prsb[:])

            # pred_e = raw + exp(lam * SM)
            esm = cst.tile([E, T], f)
            nc.scalar.activation(esm[:], sms[:], AF.Exp, scale=lam)
            peo = cst.tile([E, T], f)
            nc.vector.tensor_add(peo[:], PEH[:], esm[:])
            nc.sync.dma_start(out_pe[:], peo[:])

    nc.compile()
    return nc


_prog_cache = {}


def kernel(**inputs):
    from concourse.bass_utils import run_bass_kernel_spmd

    shared, percore, WxT, lam, br, bx = _host_prep(inputs)

    if "prog" not in _prog_cache:
        _prog_cache["prog"] = _build_program(lam, br)
    nc = _prog_cache["prog"]

    in_maps = []
    for c in range(8):
        b, vh = c // 2, c % 2
        m = dict(shared)
        m.update(percore[b])
        m["WxT"] = np.ascontiguousarray(WxT[:, vh * VH:(vh + 1) * VH]).astype(np.float16)
        in_maps.append(m)

    res = run_bass_kernel_spmd(nc, in_maps, core_ids=list(range(8)))
    globals()["LAST_EXEC_NS"] = res.exec_time_ns

    f32 = np.float32
    logits = np.empty((B, T, V), f32)
    pred_r = np.empty((B, T), f32)
    pred_e = np.empty((B, T, E), f32)
    for c in range(8):
        b, vh = c // 2, c % 2
        lo = vh * VH
        hi = min(V, lo + VH)
        if hi > lo:
            logits[b, :, lo:hi] = res.results[c]["out_logits"][:, :hi - lo]
        if vh == 0:
            pred_r[b] = res.results[c]["out_pr"][0]
            pred_e[b] = res.results[c]["out_pe"].T
    logits += bx[None, None, :]
    return logits, pred_r, pred_e


# revision 28
# speedup vs baseline: 48172.2839x; 14658.9926x over previous
"""EntityNLM forward on 8 trn2 NeuronCores.

Sharding: core c handles document b = c // 2 and vocab half vh = c % 2.
Every core runs the sequential entity-LSTM recurrence for its document
(entity indices arrive as per-core int32 offset arrays, consumed via engine
registers + dynamic access patterns), then computes its (doc, vocab-half)
slice of the logits matmul, streaming results to DRAM. Host reassembles.
No collectives.

Self-contained: shapes hardcoded; only numpy + concourse imports.
"""
import numpy as np

V, D, H, E, B, T = 50257, 300, 128, 64, 4, 512
DPAD = 384            # D padded to 3x128 for K-chunked matmuls
VPAD = 51200          # V padded to 8x6400; halves of 25600 = 50 blocks of 512
VH = VPAD // 2
NVB = VH // 512       # vocab blocks per half (50)


def _host_prep(inputs):
    """Index/layout plumbing on host (int bookkeeping + layout transforms)."""
    f32 = np.float32
    tokens = np.asarray(inputs["tokens"]).astype(np.int64)
    eid = np.asarray(inputs["entity_ids"]).astype(np.int64)
    sid = np.asarray(inputs["sent_ids"]).astype(np.float64)
    embed = np.asarray(inputs["embed"], f32)
    Wih = np.asarray(inputs["Wih"], f32)
    Whh = np.asarray(inputs["Whh"], f32)
    b2v = (np.asarray(inputs["bih"]) + np.asarray(inputs["bhh"])).astype(f32)
    We = np.asarray(inputs["We"], f32); Wd = np.asarray(inputs["Wd"], f32)
    WTe = np.asarray(inputs["WTe"], f32); WTc = np.asarray(inputs["WTc"], f32)
    Wx = np.asarray(inputs["Wx"], f32)
    e0 = np.asarray(inputs["ents0"], f32)
    lam = float(np.asarray(inputs["lam"]))

    GORD = [0, 1, 3, 2]   # torch gate order i,f,g,o -> device order i,f,o,g

    WihT = np.zeros((DPAD, 4 * H), f32)
    Wih4 = Wih.reshape(4, H, D)
    for gi, g in enumerate(GORD):
        WihT[:D, gi * H:(gi + 1) * H] = Wih4[g].T
    b2 = np.stack([b2v.reshape(4, H)[g] for g in GORD], 1)        # (H,4)

    WhhT = np.zeros((H, 4 * H), f32)
    Whh4 = Whh.reshape(4, H, H)
    for gi, g in enumerate(GORD):
        WhhT[:, gi * H:(gi + 1) * H] = Whh4[g].T

    WxT = np.zeros((H, VPAD), f32)
    WxT[:, :V] = Wx.T

    shared = dict(
        WihT=WihT, b2=b2, WhhT=WhhT,
        WeT=np.ascontiguousarray(We.T), WdT=np.ascontiguousarray(Wd.T),
        WTeT=np.ascontiguousarray(WTe.T), WTcT=np.ascontiguousarray(WTc.T),
        Wr=np.asarray(inputs["Wr"], f32).reshape(H, 1),
        ident=np.eye(H, dtype=f32),
    )

    percore = []
    for b in range(B):
        SM = np.zeros((E, T), f32)
        dists = np.zeros(E, np.float64)
        for t in range(T):
            SM[:, t] = dists - sid[b, t]
            dists[eid[b, t]] = sid[b, t]
        EtokT = np.zeros((DPAD, T), f32)
        EtokT[:D] = embed[tokens[b]].T
        percore.append(dict(
            EtokT=EtokT,
            E0C=np.ascontiguousarray(e0[b].T),
            E0R=e0[b].reshape(1, E * H).astype(f32),
            eidx=eid[b].reshape(1, T).astype(np.int32),
            eoff=(eid[b] * H).reshape(1, T).astype(np.int32),
            SM=SM,
            mask=np.broadcast_to((eid[b] > 0).astype(f32), (H, T)).copy(),
        ))

    return shared, percore, WxT, lam, float(np.asarray(inputs["br"])), \
        np.asarray(inputs["bx"], f32)


def _build_program(lam, br, nsteps=T, skip_b=False, skip_c=False):
    import concourse.bacc as bacc
    import concourse.mybir as mybir
    import concourse.tile as tile
    import concourse.bass as bass

    dt = mybir.dt
    AF = mybir.ActivationFunctionType
    OP = mybir.AluOpType

    nc = bacc.Bacc("TRN2", target_bir_lowering=False)

    def din(name, shape, dtype=dt.float32):
        return nc.dram_tensor(name, shape, dtype, kind="ExternalInput")

    EtokT = din("EtokT", [DPAD, T]); WihT = din("WihT", [DPAD, 4 * H])
    b2 = din("b2", [H, 4]); WhhT = din("WhhT", [H, 4 * H])
    WeT = din("WeT", [H, H]); WdT = din("WdT", [H, H])
    WTeT = din("WTeT", [H, H]); WTcT = din("WTcT", [H, H])
    Wr = din("Wr", [H, 1]); ident = din("ident", [H, H])
    E0C = din("E0C", [H, E]); E0R = din("E0R", [1, E * H])
    eidx = din("eidx", [1, T], dt.int32); eoff = din("eoff", [1, T], dt.int32)
    SMd = din("SM", [E, T]); maskd = din("mask", [H, T])
    WxTd = din("WxT", [H, VH], dt.float16)
    out_logits = nc.dram_tensor("out_logits", [T, VH], dt.float32,
                                kind="ExternalOutput")
    out_pr = nc.dram_tensor("out_pr", [1, T], dt.float32, kind="ExternalOutput")
    out_pe = nc.dram_tensor("out_pe", [E, T], dt.float32, kind="ExternalOutput")

    with tile.TileContext(nc) as tc:
        with (
            tc.tile_pool(name="cst", bufs=1) as cst,
            tc.tile_pool(name="loop", bufs=6) as lp,
            tc.tile_pool(name="str", bufs=3) as strm,
            tc.tile_pool(name="pg", bufs=2, space="PSUM") as pg,
            tc.tile_pool(name="prow", bufs=2, space="PSUM") as prow,
            tc.tile_pool(name="pcol", bufs=2, space="PSUM") as pcol,
            tc.tile_pool(name="pbig", bufs=2, space="PSUM") as pbig,
        ):
            f = dt.float32
            fr = dt.float32r

            def load(dram, shape, dtype=f):
                t_ = cst.tile(shape, dtype, tag=f"ld_{dram.name}")
                nc.sync.dma_start(t_[:], dram[:])
                return t_

            # persistent SBUF (DPAD-partition tensors load as (128, 3, cols))
            etok = cst.tile([H, 3 * T], f)
            nc.sync.dma_start(etok[:].rearrange("p (c t) -> p c t", c=3),
                              EtokT[:].rearrange("(c p) t -> p c t", p=H))
            wih = cst.tile([H, 3 * 4 * H], f)
            nc.sync.dma_start(wih[:].rearrange("p (c u) -> p c u", c=3),
                              WihT[:].rearrange("(c p) u -> p c u", p=H))
            b2s = load(b2, [H, 4]); whh = load(WhhT, [H, 4 * H])
            wes = load(WeT, [H, H]); wds = load(WdT, [H, H])
            wtes = load(WTeT, [H, H]); wtcs = load(WTcT, [H, H])
            wrs = load(Wr, [H, 1]); ids = load(ident, [H, H])
            bankC = load(E0C, [H, E])
            bankR = load(E0R, [1, E * H])
            eix = load(eidx, [1, T], dt.int32)
            eox = load(eoff, [1, T], dt.int32)
            sms = load(SMd, [E, T]); msk = load(maskd, [H, T])

            XP = cst.tile([H, 4 * T], f)        # col g*T + t
            H4 = cst.tile([H, T], f)
            UC = cst.tile([H, 1 + T], f)
            GC = cst.tile([H, T], f)
            PEH = cst.tile([E, T], f)
            ZC = cst.tile([H, T], f)
            cstate = cst.tile([H, 1], f)
            nc.vector.memset(cstate[:], 0.0)
            h0 = cst.tile([H, 1], f)
            nc.vector.memset(h0[:], 0.0)
            c15 = cst.tile([1, 1], f)
            nc.vector.memset(c15[:], 1.5)
            nc.scalar.copy(UC[:, 0:1], bankC[:, 0:1])

            etok_v = etok[:].rearrange("p (c t) -> p c t", c=3)
            wih_v = wih[:].rearrange("p (c u) -> p c u", c=3)

            # XP = WihT.T @ EtokT + b2, per gate
            for g in range(4):
                ps = pbig.tile([H, T], f, tag="big")
                for dc in range(3):
                    nc.tensor.matmul(
                        ps[:],
                        wih_v[:, dc, g * H:(g + 1) * H],
                        etok_v[:, dc, :],
                        start=(dc == 0), stop=(dc == 2))
                nc.scalar.activation(XP[:, g * T:(g + 1) * T], ps[:], AF.Identity,
                                     bias=b2s[:, g:g + 1])

            xp_v = XP[:].rearrange("p (g t) -> p g t", g=4)

            def emit_A(t, hprev):
                # ---- LSTM cell (order i,f,o,g)
                gps = pg.tile([H, 4], f, tag="gates")
                for g in range(4):
                    nc.tensor.matmul(gps[:, g:g + 1], whh[:, g * H:(g + 1) * H],
                                     hprev[:], start=True, stop=True)
                act = lp.tile([H, 4], f, tag="act")
                for g in range(3):
                    nc.scalar.activation(act[:, g:g + 1], gps[:, g:g + 1],
                                         AF.Sigmoid, bias=XP[:, g * T + t:g * T + t + 1])
                nc.scalar.activation(act[:, 3:4], gps[:, 3:4], AF.Tanh,
                                     bias=XP[:, 3 * T + t:3 * T + t + 1])
                m1 = lp.tile([H, 1], f, tag="m1")
                nc.vector.tensor_mul(m1[:], act[:, 0:1], act[:, 3:4])
                nc.vector.scalar_tensor_tensor(cstate[:], cstate[:], act[:, 1:2],
                                               m1[:], OP.mult, OP.add)
                tct = lp.tile([H, 1], f, tag="tc")
                nc.scalar.activation(tct[:], cstate[:], AF.Tanh)
                hcol = H4[:, t:t + 1]
                nc.vector.tensor_mul(hcol, act[:, 2:3], tct[:])
                return hcol

            def emit_B(t):
                hcol = H4[:, t:t + 1]
                # rowp: [0:H] h row, [H:2H] d row = (Wd @ h) row, via PE
                rowp = prow.tile([1, 2 * H], f, tag="rowp")
                nc.tensor.matmul(rowp[:, 0:H], hcol, ids[:], is_transpose=True,
                                 start=True, stop=True)
                nc.tensor.matmul(rowp[:, H:2 * H], hcol, wds[:],
                                 start=True, stop=True)

                rdv = nc.vector.alloc_register(f"rdv{t}")
                nc.vector.reg_load(rdv, eox[0:1, t:t + 1])
                o_dv = nc.vector.snap(rdv, donate=True, min_val=0,
                                      max_val=(E - 1) * H)
                rac = nc.scalar.alloc_register(f"rac{t}")
                nc.scalar.reg_load(rac, eix[0:1, t:t + 1])
                o_ac = nc.scalar.snap(rac, donate=True, min_val=0, max_val=E - 1)

                colp = pcol.tile([H, 4], f, tag="colp")     # _ | g | ucol | pe(64)
                nc.tensor.matmul(colp[:, 1:2], wes[:], hcol, start=True, stop=True)
                nc.vector.tensor_copy(GC[:, t:t + 1], colp[:, 1:2])

                # bank cycle, DVE-resident: dot -> sigmoid(ACT) -> blend -> norm
                draw = lp.tile([1, 1], f, tag="draw")
                dsc = lp.tile([1, H], f, tag="dsc")
                nc.vector.scalar_tensor_tensor(
                    dsc[:], bankR[0:1, bass.ds(o_dv, H)], 1.0, rowp[:, H:2 * H],
                    OP.bypass, OP.mult, accum_out=draw[:])
                delta = lp.tile([1, 1], f, tag="delta")
                nc.scalar.activation(delta[:], draw[:], AF.Sigmoid)

                diff = lp.tile([1, H], f, tag="diff")
                nc.vector.tensor_sub(diff[:], bankR[0:1, bass.ds(o_dv, H)],
                                     rowp[:, 0:H])
                upd = lp.tile([1, H], f, tag="upd")
                nc.vector.scalar_tensor_tensor(upd[:], diff[:], delta[:],
                                               rowp[:, 0:H], OP.mult, OP.add)
                sq = lp.tile([1, 1], f, tag="sq")
                sqt = lp.tile([1, H], f, tag="sqt")
                nc.vector.scalar_tensor_tensor(sqt[:], upd[:], 1.0, upd[:],
                                               OP.bypass, OP.mult,
                                               accum_out=sq[:])
                # rsqrt: bit-hack seed + 2 Newton iterations, all on DVE
                y0i = lp.tile([1, 1], dt.int32, tag="y0i")
                nc.vector.tensor_scalar(y0i[:], sq[:].bitcast(dt.int32), 1, None,
                                        OP.logical_shift_right)
                nc.vector.tensor_scalar(y0i[:], y0i[:], -1, 0x5F3759DF,
                                        OP.mult, OP.add)
                hs = lp.tile([1, 1], f, tag="hs")
                nc.vector.tensor_scalar_mul(hs[:], sq[:], -0.5)
                y = y0i[:].bitcast(f)
                for it in range(2):
                    p_ = lp.tile([1, 1], f, tag=f"ntp{it}")
                    nc.vector.tensor_mul(p_[:], y, y)
                    nc.vector.scalar_tensor_tensor(p_[:], p_[:], hs[:], c15[:],
                                                   OP.mult, OP.add)
                    yn = lp.tile([1, 1], f, tag=f"ntc{it}")
                    nc.vector.tensor_mul(yn[:], p_[:], y)
                    y = yn[:]
                updn = lp.tile([1, H], f, tag="updn")
                nc.vector.tensor_scalar_mul(updn[:], upd[:], y)
                nc.vector.tensor_copy(bankR[0:1, bass.ds(o_dv, H)], updn[:])
                nc.tensor.matmul(colp[:, 2:3], updn[:], ids[0:1, 0:1],
                                 is_transpose=True, start=True, stop=True)
                nc.scalar.copy(UC[:, 1 + t:2 + t], colp[:, 2:3])

                # pred_e raw BEFORE this step's bankC scatter
                nc.tensor.matmul(colp[0:E, 3:4], bankC[:], GC[:, t:t + 1],
                                 start=True, stop=True)
                nc.scalar.copy(PEH[:, t:t + 1], colp[0:E, 3:4])
                nc.scalar.copy(bankC[:, bass.ds(o_ac, 1)], colp[:, 2:3])

            def emit_chunk(s):
                # ---- per-chunk tails: ent_feat, Z, logits
                if True:
                    efe = pbig.tile([H, 128], f, tag="big")
                    nc.tensor.matmul(efe[:], wtes[:], UC[:, 1 + s:1 + s + 128],
                                     start=True, stop=True)
                    efc = pbig.tile([H, 128], f, tag="big")
                    nc.tensor.matmul(efc[:], wtcs[:], UC[:, s:s + 128],
                                     start=True, stop=True)
                    efcs = lp.tile([H, 128], f, tag="efcs")
                    nc.scalar.copy(efcs[:], efc[:])
                    zc = ZC[:, s:s + 128]
                    nc.vector.tensor_sub(zc, efe[:], efcs[:])
                    nc.vector.tensor_mul(zc, zc, msk[:, s:s + 128])
                    nc.vector.tensor_add(zc, zc, efcs[:])
                    nc.vector.tensor_add(zc, zc, H4[:, s:s + 128])
                    zc16 = lp.tile([H, 128], dt.float16, tag="zc16")
                    nc.vector.tensor_copy(zc16[:], zc)
                    for vb in range(NVB):
                        wx = strm.tile([H, 512], dt.float16, tag="wx")
                        nc.sync.dma_start(wx[:], WxTd[:, vb * 512:(vb + 1) * 512])
                        lps = pbig.tile([128, 512], f, tag="big")
                        nc.tensor.matmul(lps[:], zc16[:], wx[:],
                                         start=True, stop=True)
                        lsb = strm.tile([128, 512], f, tag="lsb")
                        if vb % 2 == 0:
                            nc.scalar.copy(lsb[:], lps[:])
                        else:
                            nc.vector.tensor_copy(lsb[:], lps[:])
                        nc.sync.dma_start(
                            out_logits[s:s + 128, vb * 512:(vb + 1) * 512], lsb[:])

            # software-pipelined emission: A(t) ahead of B(t-1)
            hprev = h0
            for t in range(nsteps):
                if not skip_b and t >= 1:
                    emit_B(t - 1)
                    if t % 128 == 0 and not skip_c:
                        emit_chunk(t - 128)
                hprev = emit_A(t, hprev)
            if not skip_b:
                emit_B(nsteps - 1)
            if not skip_c:
                emit_chunk(nsteps - 128)

            # pred_r
            prps = pbig.tile([1, T], f, tag="big")
            nc.tensor.matmul(prps[:], wrs[:], H4[:], start=True, stop=True)
            prsb = cst.tile([1, T], f)
            nc.scalar.activation(prsb[:], prps[:], AF.Sigmoid, bias=br)
            nc.sync.dma_start(out_pr[:], prsb[:])

            # pred_e = raw + exp(lam * SM)
            esm = cst.tile([E, T], f)
            nc.scalar.activation(esm[:], sms[:], AF.Exp, scale=lam)
            peo = cst.tile([E, T], f)
            nc.vector.tensor_add(peo[:], PEH[:], esm[:])
            nc.sync.dma_start(out_pe[:], peo[:])

    nc.compile()
    return nc


_prog_cache = {}


def kernel(**inputs):
    from concourse.bass_utils import run_bass_kernel_spmd

    shared, percore, WxT, lam, br, bx = _host_prep(inputs)

    if "prog" not in _prog_cache:
        _prog_cache["prog"] = _build_program(lam, br)
    nc = _prog_cache["prog"]

    in_maps = []
    for c in range(8):
        b, vh = c // 2, c % 2
        m = dict(shared)
        m.update(percore[b])
        m["WxT"] = np.ascontiguousarray(WxT[:, vh * VH:(vh + 1) * VH]).astype(np.float16)
        in_maps.append(m)

    res = run_bass_kernel_spmd(nc, in_maps, core_ids=list(range(8)))
    globals()["LAST_EXEC_NS"] = res.exec_time_ns

    f32 = np.float32
    logits = np.empty((B, T, V), f32)
    pred_r = np.empty((B, T), f32)
    pred_e = np.empty((B, T, E), f32)
    for c in range(8):
        b, vh = c // 2, c % 2
        lo = vh * VH
        hi = min(V, lo + VH)
        if hi > lo:
            logits[b, :, lo:hi] = res.results[c]["out_logits"][:, :hi - lo]
        if vh == 0:
            pred_r[b] = res.results[c]["out_pr"][0]
            pred_e[b] = res.results[c]["out_pe"].T
    logits += bx[None, None, :]
    return logits, pred_r, pred_e
